# revision 1
# baseline (speedup 1.0000x reference)
"""Causal self-attention (GQA + RoPE) Trainium2 Bass kernel, 8 NeuronCores.

Problem: B=2, T=2048, C=2048, n_head=16, n_kv_head=4, head_dim=128.

Sharding: 2-way batch DP x 4-way head TP. Core c = 4*b + g handles batch b,
kv head g, q heads [4g, 4g+4). wq/wk/wv column-sharded per head group, wo
row-sharded; per-core partial outputs are summed on the host (the gather /
unshard step), so no on-device collective is needed.

Device dataflow (everything transposed, fp16 matmul operands, fp32 PSUM):
  xT [C, T] resident in DRAM, streamed as [128, 512] chunks.
  QT[h] = (wqT chunk).T @ xT chunk accumulated over C    -> [128 dq, T]
  KT, VT similar.  V is re-transposed to [s, dv] chunks via PE transpose.
  RoPE applied to QT/KT in the [d, t] layout: host permutes weight rows so
  rows 0..63 are even dims, 64..127 odd dims; then
  q' = q * cos2 + swap(q) * sinn, with swap = exchange of partition halves
  (done by SBUF->SBUF DMA) and sinn = [-sin; +sin].
  Attention in S^T layout: S^T[s_blk, t] = KT_blk.T @ QT, causal mask added
  on diagonal blocks, exp on ACT (softmax max-subtraction skipped: |scores|
  is bounded ~5 so fp32 exp is safe), denominator via ones-matmul on PE,
  O^T[dv, t] accumulated per t-chunk, normalized via a K=1 broadcast matmul
  of 1/denom and a DVE multiply.
  outT_partial = woT.T @ OT accumulated over this core's 512 channels.
Host: out[b] = sum_g outT_partial[4b+g] transposed back.
"""

import sys

sys.path.insert(0, "/opt/trn_rl_repo")

import numpy as np

import concourse.bass as bass
import concourse.mybir as mybir
import concourse.tile as tile
from concourse import bacc
from concourse.bass_utils import run_bass_kernel_spmd
from concourse.masks import make_identity

F32 = mybir.dt.float32
F32R = mybir.dt.float32r
F16 = mybir.dt.float16
AF = mybir.ActivationFunctionType

B, T, C = 2, 2048, 2048
N_HEAD, N_KV_HEAD = 16, 4
HD = 128                 # head dim
QH = 4                   # q heads per core
TQ = 512                 # t-chunk (quarter of ... 2048/512 = 4 chunks)
NT = T // TQ             # 4 t-chunks
CK = C // 128            # 16 contraction chunks of 128
SCALE = 1.0 / float(np.sqrt(HD))
MASK_NEG = -1e30

_CACHE = {}


def r(ap):
    """Matmul operand tiles are already float32r-typed; identity."""
    return ap


def _build_nc():
    nc = bacc.Bacc("TRN2", target_bir_lowering=False, debug=False, num_devices=8)

    xT = nc.dram_tensor("xT", [C, T], F16, kind="ExternalInput").ap()
    wqT = nc.dram_tensor("wqT", [C, QH * HD], F16, kind="ExternalInput").ap()
    wkT = nc.dram_tensor("wkT", [C, HD], F16, kind="ExternalInput").ap()
    wvT = nc.dram_tensor("wvT", [C, HD], F16, kind="ExternalInput").ap()
    # wo pre-tiled on host: woX[co, p, h*128+d] = wo[128*co+d, 512*g+128*h+p]
    woT = nc.dram_tensor("woX", [C // 128, 128, QH * HD], F16,
                         kind="ExternalInput").ap()
    cos2 = nc.dram_tensor("cos2", [HD, T], F32, kind="ExternalInput").ap()
    sinn = nc.dram_tensor("sinn", [HD, T], F32, kind="ExternalInput").ap()
    outT = nc.dram_tensor("outT", [C, T], F32, kind="ExternalOutput").ap()

    with tile.TileContext(nc) as tc:
        _emit(nc, tc, xT, wqT, wkT, wvT, woT, cos2, sinn, outT)

    nc.compile()
    return nc


def _emit(nc, tc, xT, wqT, wkT, wvT, woT, cos2, sinn, outT):
    import contextlib

    ctx = contextlib.ExitStack()
    with ctx:
        singles = ctx.enter_context(tc.tile_pool(name="singles", bufs=1))

        # ---- resident weights and constants (fp16 matmul operands) ----
        wq_sb = singles.tile([128, CK, QH * HD], F16)
        wk_sb = singles.tile([128, CK, HD], F16)
        wv_sb = singles.tile([128, CK, HD], F16)
        for k in range(CK):
            nc.sync.dma_start(out=wq_sb[:, k, :], in_=wqT[128 * k:128 * (k + 1), :])
            nc.sync.dma_start(out=wk_sb[:, k, :], in_=wkT[128 * k:128 * (k + 1), :])
            nc.sync.dma_start(out=wv_sb[:, k, :], in_=wvT[128 * k:128 * (k + 1), :])
        cos_sb = singles.tile([HD, T], F32)
        sin_sb = singles.tile([HD, T], F32)
        nc.sync.dma_start(out=cos_sb, in_=cos2)
        nc.sync.dma_start(out=sin_sb, in_=sinn)

        ident = singles.tile([128, 128], F32)
        make_identity(nc, ident)
        # causal mask for S^T diagonal blocks: rows = s, cols = t;
        # valid (0) when s <= t, MASK_NEG when s > t.
        cmask = singles.tile([128, 128], F32)
        nc.gpsimd.memset(cmask, 0.0)
        nc.gpsimd.affine_select(
            out=cmask, in_=cmask, compare_op=mybir.AluOpType.is_ge,
            fill=MASK_NEG, base=0, pattern=[[1, 128]], channel_multiplier=-1,
        )
        # all-ones stationary: the denominator matmul ones.T @ P gives the
        # column sums replicated across all 128 PSUM partitions, i.e. the
        # denominator is produced pre-broadcast.
        ones_sq = singles.tile([128, 128], F16)
        nc.vector.memset(ones_sq, 1.0)

        # ---- activations (resident) ----
        qT_sb = singles.tile([128, QH, T], F16)    # per head [dq, t]
        kT_sb = singles.tile([128, T], F16)        # [dk, t]
        v_sb = singles.tile([128, CK, HD], F16)    # [s in chunk, (chunk, dv)]
        oT_sb = singles.tile([128, QH, T], F16)    # per head [dv, t]

        # ======== Phase B: projections, RoPE interleaved per quarter ========
        with tc.tile_pool(name="xpool", bufs=4) as xpool, \
             tc.tile_pool(name="projps", bufs=1, space="PSUM") as projps, \
             tc.tile_pool(name="vtps", bufs=1, space="PSUM") as vtps, \
             tc.tile_pool(name="vtsb", bufs=2) as vtsb, \
             tc.tile_pool(name="rope", bufs=2) as rope:
            for q in range(NT):
                t0 = TQ * q
                q_ps = [projps.tile([128, TQ], F32, tag=f"qps{_h}", name=f"q_ps{_h}")
                        for _h in range(QH)]
                k_ps = projps.tile([128, TQ], F32, tag="kps")
                v_ps = projps.tile([128, TQ], F32, tag="vps")
                for k in range(CK):
                    x_t = xpool.tile([128, TQ], F16)
                    nc.sync.dma_start(
                        out=x_t, in_=xT[128 * k:128 * (k + 1), t0:t0 + TQ])
                    st, sp = (k == 0), (k == CK - 1)
                    for h in range(QH):
                        nc.tensor.matmul(
                            q_ps[h], wq_sb[:, k, HD * h:HD * (h + 1)], x_t,
                            start=st, stop=sp)
                    nc.tensor.matmul(k_ps, wk_sb[:, k, :], x_t, start=st, stop=sp)
                    nc.tensor.matmul(v_ps, wv_sb[:, k, :], x_t, start=st, stop=sp)
                for h in range(QH):
                    nc.vector.tensor_copy(out=qT_sb[:, h, t0:t0 + TQ], in_=q_ps[h])
                nc.vector.tensor_copy(out=kT_sb[:, t0:t0 + TQ], in_=k_ps)
                # V^T [dv, 512 s] -> transpose into natural [s, dv] chunks
                vt_t = vtsb.tile([128, TQ], F32)
                nc.vector.tensor_copy(out=vt_t, in_=v_ps)
                for jj in range(TQ // 128):
                    j = 4 * q + jj
                    vt_ps = vtps.tile([128, 128], F32, tag="vtp")
                    nc.tensor.transpose(
                        vt_ps, vt_t[:, 128 * jj:128 * (jj + 1)], ident)
                    nc.vector.tensor_copy(out=v_sb[:, j, :], in_=vt_ps)
                # RoPE for this quarter on Q heads and K (overlaps next
                # quarter's projection matmuls on PE)
                for h in range(QH + 1):
                    tgt = kT_sb[:, t0:t0 + TQ] if h == QH \
                        else qT_sb[:, h, t0:t0 + TQ]
                    sw = rope.tile([128, TQ], F16, tag="swap")
                    nc.sync.dma_start(out=sw[0:64, :], in_=tgt[64:128, :])
                    nc.sync.dma_start(out=sw[64:128, :], in_=tgt[0:64, :])
                    tmp = rope.tile([128, TQ], F32, tag="tmp")
                    nc.vector.tensor_mul(tmp, tgt, cos_sb[:, t0:t0 + TQ])
                    nc.vector.tensor_mul(sw, sw, sin_sb[:, t0:t0 + TQ])
                    nc.vector.tensor_add(tgt, tmp, sw)

        # ======== Phase D/E: attention + output projection per t-chunk ======
        with tc.tile_pool(name="sps", bufs=2, space="PSUM") as sps, \
             tc.tile_pool(name="ops", bufs=2, space="PSUM") as ops, \
             tc.tile_pool(name="dps", bufs=2, space="PSUM") as dps, \
             tc.tile_pool(name="outps", bufs=2, space="PSUM") as outps, \
             tc.tile_pool(name="ppool", bufs=5) as ppool, \
             tc.tile_pool(name="isb", bufs=2) as isb, \
             tc.tile_pool(name="wopool", bufs=3) as wopool, \
             tc.tile_pool(name="outsb", bufs=3) as outsb:
            for i in range(NT):
                ti = TQ * i
                for h in range(QH):
                    o_ps = ops.tile([128, TQ], F32, tag="o")
                    den_ps = dps.tile([128, TQ], F32, tag="d")
                    nj = 4 * (i + 1)
                    for j in range(nj):
                        t0 = max(ti, 128 * j)
                        N = TQ * (i + 1) - t0
                        c0 = t0 - ti        # col offset in this t-chunk
                        s_ps = sps.tile([128, TQ], F32, tag="s")
                        nc.tensor.matmul(
                            s_ps[:, :N],
                            kT_sb[:, 128 * j:128 * (j + 1)],
                            qT_sb[:, h, t0:t0 + N],
                            start=True, stop=True)
                        if j >= 4 * i:  # diagonal block sits at cols [0,128)
                            nc.vector.tensor_add(
                                s_ps[:, 0:128], s_ps[:, 0:128], cmask)
                        p_t = ppool.tile([128, TQ], F16, tag="p")
                        nc.scalar.activation(
                            p_t[:, :N], s_ps[:, :N], AF.Exp, scale=SCALE)
                        st, sp = (j == 0), (j == nj - 1)
                        nc.tensor.matmul(
                            den_ps[:, c0:c0 + N], ones_sq, p_t[:, :N],
                            start=st, stop=sp)
                        nc.tensor.matmul(
                            o_ps[:, c0:c0 + N], v_sb[:, j, :], p_t[:, :N],
                            start=st, stop=sp)
                    inv_t = isb.tile([128, TQ], F32, tag="inv")
                    nc.vector.reciprocal(inv_t, den_ps)
                    nc.vector.tensor_mul(oT_sb[:, h, ti:ti + TQ], o_ps, inv_t)
                # output projection for this t-chunk
                for co in range(C // 128):
                    wo_t = wopool.tile([128, QH, 128], F16, tag="wo")
                    nc.sync.dma_start(
                        out=wo_t[:, :, :],
                        in_=woT[co].rearrange("p (h d) -> p h d", h=QH))
                    ot_ps = outps.tile([128, TQ], F32, tag="op")
                    for h in range(QH):
                        nc.tensor.matmul(
                            ot_ps, wo_t[:, h, :], oT_sb[:, h, ti:ti + TQ],
                            start=(h == 0), stop=(h == QH - 1))
                    out_t = outsb.tile([128, TQ], F32, tag="outt")
                    nc.vector.tensor_copy(out=out_t, in_=ot_ps)
                    nc.sync.dma_start(
                        out=outT[128 * co:128 * (co + 1), ti:ti + TQ],
                        in_=out_t)


_PERM = np.concatenate([np.arange(0, HD, 2), np.arange(1, HD, 2)])

PROFILE = False
LAST_EXEC_NS = None
LAST_RESULTS = None


def kernel(x, freqs_cos, freqs_sin, wq, wk, wv, wo):
    global LAST_EXEC_NS, LAST_RESULTS
    if "nc" not in _CACHE:
        _CACHE["nc"] = _build_nc()
    nc = _CACHE["nc"]

    x = np.asarray(x, dtype=np.float32)
    fc = np.asarray(freqs_cos, dtype=np.float32)
    fs = np.asarray(freqs_sin, dtype=np.float32)
    wq = np.asarray(wq, dtype=np.float32)
    wk = np.asarray(wk, dtype=np.float32)
    wv = np.asarray(wv, dtype=np.float32)
    wo = np.asarray(wo, dtype=np.float32)

    cosT = fc.T                                   # [64, T]
    sinT = fs.T
    cos2 = np.ascontiguousarray(np.concatenate([cosT, cosT], axis=0))  # [128,T]
    sinn = np.ascontiguousarray(np.concatenate([-sinT, sinT], axis=0))

    in_maps = []
    for core in range(8):
        b, g = core // 4, core % 4
        xTb = np.ascontiguousarray(x[b].T.astype(np.float16))    # [C, T]
        wq_g = wq[512 * g:512 * (g + 1)].reshape(QH, HD, C)[:, _PERM, :]
        wqT = np.ascontiguousarray(
            wq_g.reshape(QH * HD, C).T.astype(np.float16))       # [C, 512]
        wkT = np.ascontiguousarray(
            wk[HD * g:HD * (g + 1)][_PERM].T.astype(np.float16))  # [C, 128]
        wvT = np.ascontiguousarray(
            wv[HD * g:HD * (g + 1)].T.astype(np.float16))         # [C, 128]
        wo_g = wo[:, 512 * g:512 * (g + 1)]                      # [C, 512]
        woX = np.ascontiguousarray(
            wo_g.reshape(16, 128, QH, 128).transpose(0, 3, 2, 1)
        ).astype(np.float16).reshape(16, 128, QH * 128)          # [16,128,512]
        in_maps.append({
            "xT": xTb, "wqT": wqT, "wkT": wkT, "wvT": wvT, "woX": woX,
            "cos2": cos2, "sinn": sinn,
        })

    res = run_bass_kernel_spmd(nc, in_maps, list(range(8)), trace=PROFILE)
    LAST_EXEC_NS = res.exec_time_ns
    LAST_RESULTS = res

    out = np.empty((B, T, C), dtype=np.float32)
    for b in range(B):
        acc = res.results[4 * b]["outT"].astype(np.float32)
        for g in range(1, 4):
            acc = acc + res.results[4 * b + g]["outT"]
        out[b] = acc.T
    return out



# revision 4
# speedup vs baseline: 1.1268x; 1.1268x over previous
"""Causal self-attention (GQA + RoPE) Trainium2 Bass kernel, 8 NeuronCores.

Problem: B=2, T=2048, C=2048, n_head=16, n_kv_head=4, head_dim=128.

Sharding: 2-way batch DP x 4-way head TP. Core c = 4*b + g handles batch b,
kv head g, q heads [4g, 4g+4). wq/wk/wv column-sharded per head group, wo
row-sharded; per-core partial outputs are summed on the host.

Device dataflow (fp16 matmul operands, fp32 PSUM), fused per-quarter
pipeline  PROJ(q) -> WO(q-1) -> ATT(q)  so the PE never drains:

  PROJ(q):  Q^T/K^T/V^T [d, 512 t] accumulated over C in 16 chunks from a
            resident x-quarter tile; V re-transposed to [s, dv] via PE;
            RoPE applied in [d, t] layout (swap halves via SBUF->SBUF DMA).
  ATT(q):   S^T[s_blk, t] = K^T_blk.T @ Q^T per 128-s-block; causal mask on
            diagonal blocks added by an extra matmul with precomputed
            triangular factors (ramp trick, no DVE on the critical path);
            exp on ACT over a [128, <=1024] wide tile (2 s-blocks/op);
            denominator via ones-matmul; O^T accumulated per head;
            1/den via vector.reciprocal_approx_fast; O^T normalized on DVE.
  WO(q):    out^T partial [128 rows, 512 t] = sum_h woX.T @ O^T, drained
            alternately on ACT/DVE into an out tile, one big DMA per quarter.

PSUM plan (8 banks): tag A = 2 x [128,1024] (proj q-head pair accumulators /
wide score tiles), tag B = 2 x [128,512] (k/v accs, o accs, wo accs),
tag C = 2 x [128,512] (v-transpose tiles, denominator accs). Ring reuse of
tags encodes the pipeline dependencies.

All DRAM tensors are laid out host-side so every DMA moves >=4KB contiguous
per partition (the baseline's 1KB rows were descriptor-rate limited).
"""

import sys

sys.path.insert(0, "/opt/trn_rl_repo")

import numpy as np

import concourse.bass as bass
import concourse.mybir as mybir
import concourse.tile as tile
from concourse import bacc
from concourse.bass_utils import run_bass_kernel_spmd
from concourse.masks import make_identity

F32 = mybir.dt.float32
F16 = mybir.dt.float16
AF = mybir.ActivationFunctionType

B, T, C = 2, 2048, 2048
N_HEAD, N_KV_HEAD = 16, 4
HD = 128                 # head dim
QH = 4                   # q heads per core
TQ = 512                 # t-chunk
NQ = T // TQ             # 4 quarters
CK = C // 128            # 16 contraction chunks of 128
SCALE = 1.0 / float(np.sqrt(HD))
MASK_NEG = -30000.0

_CACHE = {}


def _build_nc():
    nc = bacc.Bacc("TRN2", target_bir_lowering=False, debug=False, num_devices=8)

    xQ = nc.dram_tensor("xQ", [128, NQ, CK, TQ], F16, kind="ExternalInput").ap()
    wqX = nc.dram_tensor("wqX", [128, CK, QH * HD], F16, kind="ExternalInput").ap()
    wkX = nc.dram_tensor("wkX", [128, CK, HD], F16, kind="ExternalInput").ap()
    wvX = nc.dram_tensor("wvX", [128, CK, HD], F16, kind="ExternalInput").ap()
    woX = nc.dram_tensor("woX", [128, CK, QH, HD], F16, kind="ExternalInput").ap()
    cosX = nc.dram_tensor("cosX", [HD, T], F16, kind="ExternalInput").ap()
    sinX = nc.dram_tensor("sinX", [HD, T], F16, kind="ExternalInput").ap()
    outX = nc.dram_tensor("outX", [128, NQ, CK, TQ], F16, kind="ExternalOutput").ap()

    with tile.TileContext(nc) as tc:
        _emit(nc, tc, xQ, wqX, wkX, wvX, woX, cosX, sinX, outX)

    nc.compile()
    return nc


def _emit(nc, tc, xQ, wqX, wkX, wvX, woX, cosX, sinX, outX):
    import contextlib

    ctx = contextlib.ExitStack()
    with ctx:
        singles = ctx.enter_context(tc.tile_pool(name="singles", bufs=1))
        psum = ctx.enter_context(tc.tile_pool(name="ps", bufs=1, space="PSUM"))
        xring = ctx.enter_context(tc.tile_pool(name="xr", bufs=2))
        ppool = ctx.enter_context(tc.tile_pool(name="pp", bufs=4))
        rpool = ctx.enter_context(tc.tile_pool(name="rp", bufs=4))
        vtpool = ctx.enter_context(tc.tile_pool(name="vtp", bufs=2))
        outsb = ctx.enter_context(tc.tile_pool(name="ou", bufs=2))

        # ---- resident weights / activations ----
        wq_sb = singles.tile([128, CK, QH * HD], F16)
        wk_sb = singles.tile([128, CK, HD], F16)
        wv_sb = singles.tile([128, CK, HD], F16)
        wo_sb = singles.tile([128, CK, QH, HD], F16)
        cos_sb = singles.tile([HD, T], F16)
        sin_sb = singles.tile([HD, T], F16)
        qT = [singles.tile([128, QH, TQ], F16, name=f"qT{q}") for q in range(NQ)]
        kT = [singles.tile([128, TQ], F16, name=f"kT{q}") for q in range(NQ)]
        vS = [singles.tile([128, 4, HD], F16, name=f"vS{q}") for q in range(NQ)]
        oT = [singles.tile([128, QH, TQ], F16, name=f"oT{q}") for q in range(NQ)]

        # ---- input DMAs, in startup-critical order ----
        for pp4 in range(4):
            nc.sync.dma_start(out=wq_sb[:, 4 * pp4:4 * pp4 + 4, :],
                              in_=wqX[:, 4 * pp4:4 * pp4 + 4, :])
        nc.sync.dma_start(out=wk_sb, in_=wkX)
        nc.sync.dma_start(out=wv_sb, in_=wvX)
        xt = [None] * NQ
        xt[0] = xring.tile([128, CK, TQ], F16, tag="x", name="xt0")
        nc.sync.dma_start(out=xt[0], in_=xQ[:, 0, :, :])
        nc.sync.dma_start(out=cos_sb, in_=cosX)
        nc.sync.dma_start(out=sin_sb, in_=sinX)
        xt[1] = xring.tile([128, CK, TQ], F16, tag="x", name="xt1")
        nc.sync.dma_start(out=xt[1], in_=xQ[:, 1, :, :])
        nc.sync.dma_start(out=wo_sb, in_=woX)

        # ---- constants ----
        ident = singles.tile([128, 128], F32)
        make_identity(nc, ident)
        ones_sq = singles.tile([128, 128], F16)
        nc.vector.memset(ones_sq, 1.0)
        # causal ramp factors: mask[s,t'] = sum_r A[r,s]*Bm[r,t']
        #   = MASK_NEG * (s - t') for s > t', 0 otherwise  (t' = col in diag blk)
        maskA = singles.tile([128, 128], F16)
        nc.gpsimd.memset(maskA, 1.0)
        # keep 1 where s - r - 1 >= 0 (r < s), else 0
        nc.gpsimd.affine_select(
            out=maskA, in_=maskA, compare_op=mybir.AluOpType.is_ge,
            fill=0.0, base=-1, pattern=[[1, 128]], channel_multiplier=-1)
        maskB = singles.tile([128, 128], F16)
        nc.gpsimd.memset(maskB, MASK_NEG)
        # keep MASK_NEG where r - t' >= 0 (r >= t'), else 0
        nc.gpsimd.affine_select(
            out=maskB, in_=maskB, compare_op=mybir.AluOpType.is_ge,
            fill=0.0, base=0, pattern=[[-1, 128]], channel_multiplier=1)

        def proj(q):
            q01 = psum.tile([128, 2 * TQ], F32, tag="A", name=f"q01_{q}")
            q23 = psum.tile([128, 2 * TQ], F32, tag="A", name=f"q23_{q}")
            kacc = psum.tile([128, TQ], F32, tag="B", name=f"kacc{q}")
            vacc = psum.tile([128, TQ], F32, tag="B", name=f"vacc{q}")
            for k in range(CK):
                xk = xt[q][:, k, :]
                st, sp = (k == 0), (k == CK - 1)
                nc.tensor.matmul(q01[:, 0:TQ], wq_sb[:, k, 0:128], xk,
                                 start=st, stop=sp)
                nc.tensor.matmul(q01[:, TQ:2 * TQ], wq_sb[:, k, 128:256], xk,
                                 start=st, stop=sp)
                nc.tensor.matmul(q23[:, 0:TQ], wq_sb[:, k, 256:384], xk,
                                 start=st, stop=sp)
                nc.tensor.matmul(q23[:, TQ:2 * TQ], wq_sb[:, k, 384:512], xk,
                                 start=st, stop=sp)
                nc.tensor.matmul(kacc, wk_sb[:, k, :], xk, start=st, stop=sp)
                nc.tensor.matmul(vacc, wv_sb[:, k, :], xk, start=st, stop=sp)
            # V: drain fp32, PE-transpose to natural [s, dv]
            vt = vtpool.tile([128, TQ], F32, tag="v")
            nc.vector.tensor_copy(out=vt, in_=vacc)
            for jj in range(4):
                vtp = psum.tile([128, 128], F32, tag="C", name=f"vtp{q}_{jj}")
                nc.tensor.transpose(vtp, vt[:, 128 * jj:128 * (jj + 1)], ident)
                nc.scalar.copy(out=vS[q][:, jj, :], in_=vtp)
            # Q/K drains: q0,q1 on ACT; q2,q3,k on DVE
            nc.scalar.copy(out=qT[q][:, 0, :], in_=q01[:, 0:TQ])
            nc.scalar.copy(out=qT[q][:, 1, :], in_=q01[:, TQ:2 * TQ])
            nc.vector.tensor_copy(out=qT[q][:, 2, :], in_=q23[:, 0:TQ])
            nc.vector.tensor_copy(out=qT[q][:, 3, :], in_=q23[:, TQ:2 * TQ])
            nc.vector.tensor_copy(out=kT[q], in_=kacc)
            # RoPE in [d, t] layout: rows 0..63 even dims, 64..127 odd dims.
            cs = cos_sb[:, TQ * q:TQ * (q + 1)]
            sn = sin_sb[:, TQ * q:TQ * (q + 1)]
            tgts = [qT[q][:, h, :] for h in range(QH)] + [kT[q]]
            sws = []
            for tgt in tgts:  # issue all swap DMAs first (latency overlap)
                sw = rpool.tile([128, TQ], F16, tag="sw", bufs=5)
                nc.sync.dma_start(out=sw[0:64, :], in_=tgt[64:128, :])
                nc.sync.dma_start(out=sw[64:128, :], in_=tgt[0:64, :])
                sws.append(sw)
            for tgt, sw in zip(tgts, sws):
                tmp = rpool.tile([128, TQ], F16, tag="tmp")
                nc.vector.tensor_mul(tmp, tgt, cs)
                nc.vector.tensor_mul(sw, sw, sn)
                nc.vector.tensor_add(tgt, tmp, sw)

        def att(q):
            nj = 4 * (q + 1)
            L = nj // 2  # wide steps per head

            def blk(j):
                c0 = max(0, 128 * j - TQ * q)
                return c0, TQ - c0, j >= 4 * q

            def emit_swide(h, step):
                s_t = psum.tile([128, 2 * TQ], F32, tag="A",
                                name=f"s{q}_{h}_{step}")
                for u in (0, 1):
                    j = 2 * step + u
                    c0, N, diag = blk(j)
                    nc.tensor.matmul(
                        s_t[:, TQ * u:TQ * u + N],
                        kT[j // 4][:, 128 * (j % 4):128 * (j % 4) + 128],
                        qT[q][:, h, c0:TQ],
                        start=True, stop=not diag)
                    if diag:
                        nc.tensor.matmul(
                            s_t[:, TQ * u:TQ * u + 128], maskA, maskB,
                            start=False, stop=True)
                p_t = ppool.tile([128, 2 * TQ], F16, tag="p",
                                 name=f"p{q}_{h}_{step}")
                _, N1, _ = blk(2 * step + 1)
                if step == 0:
                    # split so the first den/o of this head waits less
                    nc.scalar.activation(
                        p_t[:, 0:TQ], s_t[:, 0:TQ], AF.Exp, scale=SCALE)
                    nc.scalar.activation(
                        p_t[:, TQ:TQ + N1], s_t[:, TQ:TQ + N1], AF.Exp,
                        scale=SCALE)
                else:
                    nc.scalar.activation(
                        p_t[:, 0:TQ + N1], s_t[:, 0:TQ + N1], AF.Exp,
                        scale=SCALE)
                return p_t

            def emit_deno(h, step, oacc, dacc, p_t):
                for u in (0, 1):
                    j = 2 * step + u
                    c0, N, _ = blk(j)
                    st, sp = (j == 0), (j == nj - 1)
                    nc.tensor.matmul(dacc[:, c0:c0 + N], ones_sq,
                                     p_t[:, TQ * u:TQ * u + N],
                                     start=st, stop=sp)
                    nc.tensor.matmul(oacc[:, c0:c0 + N],
                                     vS[j // 4][:, j % 4, :],
                                     p_t[:, TQ * u:TQ * u + N],
                                     start=st, stop=sp)

            def normalize(h, oacc, dacc):
                inv = rpool.tile([128, TQ], F32, tag="inv")
                nc.vector.reciprocal_approx_fast(out=inv, in_=dacc)
                nc.vector.tensor_mul(oT[q][:, h, :], oacc, inv)

            # flattened pipeline over both head pairs: den/o lags s/exp by one
            # position, so pair 2's prologue covers pair 1's epilogue.
            seq = [(h, step) for h in range(QH) for step in range(L)]
            state = {}  # h -> (oacc, dacc)
            pend = []   # [(h, step, p_t)]
            for (h, step) in seq:
                if step == 0:
                    oacc = psum.tile([128, TQ], F32, tag="B", name=f"o{q}_{h}")
                    dacc = psum.tile([128, TQ], F32, tag="C", name=f"d{q}_{h}")
                    state[h] = (oacc, dacc)
                p_t = emit_swide(h, step)
                pend.append((h, step, p_t))
                if len(pend) > 1:
                    ph, pstep, pp = pend.pop(0)
                    emit_deno(ph, pstep, *state[ph], pp)
                    if pstep == L - 1:
                        normalize(ph, *state[ph])
            ph, pstep, pp = pend.pop(0)
            emit_deno(ph, pstep, *state[ph], pp)
            normalize(ph, *state[ph])

        def wo_proj(q):
            out_t = outsb.tile([128, CK, TQ], F16, tag="o", name=f"out{q}")
            for co in range(CK):
                op = psum.tile([128, TQ], F32, tag="B", name=f"op{q}_{co}")
                for h in range(QH):
                    nc.tensor.matmul(op, wo_sb[:, co, h, :], oT[q][:, h, :],
                                     start=(h == 0), stop=(h == QH - 1))
                if co % 2 == 0:
                    nc.scalar.copy(out=out_t[:, co, :], in_=op)
                else:
                    nc.vector.tensor_copy(out=out_t[:, co, :], in_=op)
            nc.gpsimd.dma_start(out=outX[:, q, :, :], in_=out_t)

        for q in range(NQ):
            proj(q)
            if q + 2 < NQ:
                xt[q + 2] = xring.tile([128, CK, TQ], F16, tag="x",
                                       name=f"xt{q + 2}")
                nc.sync.dma_start(out=xt[q + 2], in_=xQ[:, q + 2, :, :])
            if q > 0:
                wo_proj(q - 1)
            att(q)
        wo_proj(NQ - 1)


_PERM = np.concatenate([np.arange(0, HD, 2), np.arange(1, HD, 2)])

PROFILE = False
LAST_EXEC_NS = None
LAST_RESULTS = None


def kernel(x, freqs_cos, freqs_sin, wq, wk, wv, wo):
    global LAST_EXEC_NS, LAST_RESULTS
    if "nc" not in _CACHE:
        _CACHE["nc"] = _build_nc()
    nc = _CACHE["nc"]

    x = np.asarray(x, dtype=np.float32)
    fc = np.asarray(freqs_cos, dtype=np.float32)
    fs = np.asarray(freqs_sin, dtype=np.float32)
    wq = np.asarray(wq, dtype=np.float32)
    wk = np.asarray(wk, dtype=np.float32)
    wv = np.asarray(wv, dtype=np.float32)
    wo = np.asarray(wo, dtype=np.float32)

    cosT = fc.T                                   # [64, T]
    sinT = fs.T
    cosX = np.ascontiguousarray(
        np.concatenate([cosT, cosT], axis=0)).astype(np.float16)   # [128, T]
    sinX = np.ascontiguousarray(
        np.concatenate([-sinT, sinT], axis=0)).astype(np.float16)

    in_maps = []
    for core in range(8):
        b, g = core // 4, core % 4
        # x: [p, q, k, t] = x[b, 512q+t, 128k+p]
        xb = x[b].astype(np.float16)              # [T, C]
        xQd = np.ascontiguousarray(
            xb.reshape(NQ, TQ, CK, 128).transpose(3, 0, 2, 1))
        # wq (rope-permuted rows): wqX[p, k, d] = wq_g.T[128k+p, d]
        wq_g = wq[512 * g:512 * (g + 1)].reshape(QH, HD, C)[:, _PERM, :]
        wqT = wq_g.reshape(QH * HD, C).T.astype(np.float16)        # [C, 512]
        wqX = np.ascontiguousarray(
            wqT.reshape(CK, 128, QH * HD).transpose(1, 0, 2))
        wkT = wk[HD * g:HD * (g + 1)][_PERM].T.astype(np.float16)  # [C, 128]
        wkX = np.ascontiguousarray(wkT.reshape(CK, 128, HD).transpose(1, 0, 2))
        wvT = wv[HD * g:HD * (g + 1)].T.astype(np.float16)
        wvX = np.ascontiguousarray(wvT.reshape(CK, 128, HD).transpose(1, 0, 2))
        # wo: woX[p, co, h, d] = wo[128co+d, 512g+128h+p]
        wo_g = wo[:, 512 * g:512 * (g + 1)].astype(np.float16)     # [C, 512]
        woX = np.ascontiguousarray(
            wo_g.reshape(CK, 128, QH, 128).transpose(3, 0, 2, 1))
        in_maps.append({
            "xQ": xQd, "wqX": wqX, "wkX": wkX, "wvX": wvX, "woX": woX,
            "cosX": cosX, "sinX": sinX,
        })

    res = run_bass_kernel_spmd(nc, in_maps, list(range(8)), trace=PROFILE)
    LAST_EXEC_NS = res.exec_time_ns
    LAST_RESULTS = res

    out = np.empty((B, T, C), dtype=np.float32)
    for b in range(B):
        acc = res.results[4 * b]["outX"].astype(np.float32)
        for g in range(1, 4):
            acc = acc + res.results[4 * b + g]["outX"]
        # outX[d, q, co, t] -> out[512q+t, 128co+d]
        out[b] = acc.transpose(1, 3, 2, 0).reshape(T, C)
    return out


# revision 10
# speedup vs baseline: 1.4821x; 1.3153x over previous
"""Causal self-attention (GQA + RoPE) Trainium2 Bass kernel, 8 NeuronCores.

Problem: B=2, T=2048, C=2048, n_head=16, n_kv_head=4, head_dim=128.

Sharding: 2-way batch DP x 4-way head TP. Core c = 4*b + g handles batch b,
kv head g, q heads [4g, 4g+4). wq/wk/wv column-sharded per head group, wo
row-sharded; per-core partial outputs are summed on the host.

Device dataflow (fp16 matmul operands, fp32 PSUM), fused per-quarter
pipeline  PROJ(q) -> WO(q-1) -> ATT(q)  so the PE never drains:

  PROJ(q):  Q^T/K^T/V^T [d, 512 t] accumulated over C in 16 chunks from a
            resident x-quarter tile; V re-transposed to [s, dv] via PE;
            RoPE applied in [d, t] layout (swap halves via SBUF->SBUF DMA).
  ATT(q):   S^T[s_blk, t] = K^T_blk.T @ Q^T per 128-s-block; causal mask on
            diagonal blocks added by an extra matmul with precomputed
            triangular factors (ramp trick, no DVE on the critical path);
            exp on ACT over a [128, <=1024] wide tile (2 s-blocks/op);
            denominator via ones-matmul; O^T accumulated per head;
            1/den via vector.reciprocal_approx_fast; O^T normalized on DVE.
  WO(q):    out^T partial [128 rows, 512 t] = sum_h woX.T @ O^T, drained
            alternately on ACT/DVE into an out tile, one big DMA per quarter.

PSUM plan (8 banks): tag A = 2 x [128,1024] (proj q-head pair accumulators /
wide score tiles), tag B = 2 x [128,512] (k/v accs, o accs, wo accs),
tag C = 2 x [128,512] (v-transpose tiles, denominator accs). Ring reuse of
tags encodes the pipeline dependencies.

All DRAM tensors are laid out host-side so every DMA moves >=4KB contiguous
per partition (the baseline's 1KB rows were descriptor-rate limited).
"""

import sys

sys.path.insert(0, "/opt/trn_rl_repo")

import numpy as np

import concourse.bass as bass
import concourse.mybir as mybir
import concourse.tile as tile
from concourse import bacc
from concourse.bass_utils import run_bass_kernel_spmd
from concourse.masks import make_identity

F32 = mybir.dt.float32
F16 = mybir.dt.float16
AF = mybir.ActivationFunctionType

B, T, C = 2, 2048, 2048
N_HEAD, N_KV_HEAD = 16, 4
HD = 128                 # head dim
QH = 4                   # q heads per core
TQ = 512                 # t-chunk
NQ = T // TQ             # 4 quarters
CK = C // 128            # 16 contraction chunks of 128
SCALE = 1.0 / float(np.sqrt(HD))
MASK_NEG = -30000.0

_CACHE = {}


def _build_nc():
    nc = bacc.Bacc("TRN2", target_bir_lowering=False, debug=False, num_devices=8)

    xQ = nc.dram_tensor("xQ", [128, NQ, CK, TQ], F16, kind="ExternalInput").ap()
    wqX = nc.dram_tensor("wqX", [128, CK, QH * HD], F16, kind="ExternalInput").ap()
    wkX = nc.dram_tensor("wkX", [128, CK, HD], F16, kind="ExternalInput").ap()
    wvX = nc.dram_tensor("wvX", [128, CK, HD], F16, kind="ExternalInput").ap()
    woX = nc.dram_tensor("woX", [128, CK, QH, HD], F16, kind="ExternalInput").ap()
    cosX = nc.dram_tensor("cosX", [HD, T], F16, kind="ExternalInput").ap()
    sinX = nc.dram_tensor("sinX", [HD, T], F16, kind="ExternalInput").ap()
    outX = nc.dram_tensor("outX", [128, NQ, CK, TQ], F16, kind="ExternalOutput").ap()

    with tile.TileContext(nc) as tc:
        _emit(nc, tc, xQ, wqX, wkX, wvX, woX, cosX, sinX, outX)

    nc.compile()
    return nc


def _emit(nc, tc, xQ, wqX, wkX, wvX, woX, cosX, sinX, outX):
    import contextlib

    ctx = contextlib.ExitStack()
    with ctx:
        singles = ctx.enter_context(tc.tile_pool(name="singles", bufs=1))
        psum = ctx.enter_context(tc.tile_pool(name="ps", bufs=1, space="PSUM"))
        xring = ctx.enter_context(tc.tile_pool(name="xr", bufs=2))
        ppool = ctx.enter_context(tc.tile_pool(name="pp", bufs=4))
        rpool = ctx.enter_context(tc.tile_pool(name="rp", bufs=4))
        vtpool = ctx.enter_context(tc.tile_pool(name="vtp", bufs=2))
        outsb = ctx.enter_context(tc.tile_pool(name="ou", bufs=2))

        # ---- resident weights / activations ----
        wq_sb = singles.tile([128, CK, QH * HD], F16)
        wk_sb = singles.tile([128, CK, HD], F16)
        wv_sb = singles.tile([128, CK, HD], F16)
        wo_sb = singles.tile([128, CK, QH, HD], F16)
        cos_sb = singles.tile([HD, T], F16)
        sin_sb = singles.tile([HD, T], F16)
        qT = [singles.tile([128, QH, TQ], F16, name=f"qT{q}") for q in range(NQ)]
        kT = [singles.tile([128, TQ], F16, name=f"kT{q}") for q in range(NQ)]
        vS = [singles.tile([128, 4, HD], F16, name=f"vS{q}") for q in range(NQ)]
        oT = [singles.tile([128, QH, TQ], F16, name=f"oT{q}") for q in range(NQ)]

        # ---- input DMAs, in startup-critical order ----
        # x quarters as half-tiles (chunks 0-7 / 8-15) so PROJ(0) can start
        # after ~2MB of traffic instead of ~5MB.
        xt = [None] * NQ

        def x_load(q):
            a = xring.tile([128, CK // 2, TQ], F16, tag="x", bufs=4,
                           name=f"xt{q}a")
            b = xring.tile([128, CK // 2, TQ], F16, tag="x", bufs=4,
                           name=f"xt{q}b")
            nc.sync.dma_start(out=a, in_=xQ[:, q, 0:CK // 2, :])
            nc.sync.dma_start(out=b, in_=xQ[:, q, CK // 2:CK, :])
            xt[q] = (a, b)

        nc.sync.dma_start(out=wq_sb[:, 0:4, :], in_=wqX[:, 0:4, :])
        nc.sync.dma_start(out=wk_sb, in_=wkX)
        nc.sync.dma_start(out=wv_sb, in_=wvX)
        nc.sync.dma_start(out=wq_sb[:, 4:8, :], in_=wqX[:, 4:8, :])
        x_load(0)
        nc.sync.dma_start(out=wq_sb[:, 8:12, :], in_=wqX[:, 8:12, :])
        nc.sync.dma_start(out=wq_sb[:, 12:16, :], in_=wqX[:, 12:16, :])
        nc.sync.dma_start(out=cos_sb, in_=cosX)
        nc.sync.dma_start(out=sin_sb, in_=sinX)
        x_load(1)
        nc.sync.dma_start(out=wo_sb, in_=woX)

        # ---- constants ----
        ident = singles.tile([128, 128], F16)
        make_identity(nc, ident)
        ones_sq = singles.tile([128, 128], F16)
        nc.vector.memset(ones_sq, 1.0)
        # causal ramp factors: mask[s,t'] = sum_r A[r,s]*Bm[r,t']
        #   = MASK_NEG * (s - t') for s > t', 0 otherwise  (t' = col in diag blk)
        maskA = singles.tile([128, 128], F16)
        nc.gpsimd.memset(maskA, 1.0)
        # keep 1 where s - r - 1 >= 0 (r < s), else 0
        nc.gpsimd.affine_select(
            out=maskA, in_=maskA, compare_op=mybir.AluOpType.is_ge,
            fill=0.0, base=-1, pattern=[[1, 128]], channel_multiplier=-1)
        maskB = singles.tile([128, 128], F16)
        nc.gpsimd.memset(maskB, MASK_NEG)
        # keep MASK_NEG where r - t' >= 0 (r >= t'), else 0
        nc.gpsimd.affine_select(
            out=maskB, in_=maskB, compare_op=mybir.AluOpType.is_ge,
            fill=0.0, base=0, pattern=[[-1, 128]], channel_multiplier=1)

        def proj(q):
            q01 = psum.tile([128, 2 * TQ], F32, tag="A", bufs=2, name=f"q01_{q}")
            q23 = psum.tile([128, 2 * TQ], F32, tag="A", bufs=2, name=f"q23_{q}")
            kacc = psum.tile([128, TQ], F32, tag="B", bufs=2, name=f"kacc{q}")
            vacc = psum.tile([128, TQ], F32, tag="B", bufs=2, name=f"vacc{q}")
            for k in range(CK):
                xk = xt[q][k // 8][:, k % 8, :]
                st, sp = (k == 0), (k == CK - 1)
                nc.tensor.matmul(q01[:, 0:TQ], wq_sb[:, k, 0:128], xk,
                                 start=st, stop=sp)
                nc.tensor.matmul(q01[:, TQ:2 * TQ], wq_sb[:, k, 128:256], xk,
                                 start=st, stop=sp)
                nc.tensor.matmul(q23[:, 0:TQ], wq_sb[:, k, 256:384], xk,
                                 start=st, stop=sp)
                nc.tensor.matmul(q23[:, TQ:2 * TQ], wq_sb[:, k, 384:512], xk,
                                 start=st, stop=sp)
                nc.tensor.matmul(kacc, wk_sb[:, k, :], xk, start=st, stop=sp)
                nc.tensor.matmul(vacc, wv_sb[:, k, :], xk, start=st, stop=sp)
            # V: drain fp32, PE-transpose to natural [s, dv]
            vt = vtpool.tile([128, TQ], F16, tag="v")
            nc.vector.tensor_copy(out=vt, in_=vacc)
            for jj in range(4):
                vtp = psum.tile([128, 128], F16, tag="C", bufs=2, name=f"vtp{q}_{jj}")
                nc.tensor.transpose(vtp, vt[:, 128 * jj:128 * (jj + 1)], ident)
                nc.scalar.copy(out=vS[q][:, jj, :], in_=vtp)
            # Q/K drains: q0,q1 on ACT; q2,q3,k on DVE
            nc.scalar.copy(out=qT[q][:, 0, :], in_=q01[:, 0:TQ])
            nc.scalar.copy(out=qT[q][:, 1, :], in_=q01[:, TQ:2 * TQ])
            nc.vector.tensor_copy(out=qT[q][:, 2, :], in_=q23[:, 0:TQ])
            nc.vector.tensor_copy(out=qT[q][:, 3, :], in_=q23[:, TQ:2 * TQ])
            nc.vector.tensor_copy(out=kT[q], in_=kacc)
            # RoPE in [d, t] layout: rows 0..63 even dims, 64..127 odd dims.
            cs = cos_sb[:, TQ * q:TQ * (q + 1)]
            sn = sin_sb[:, TQ * q:TQ * (q + 1)]
            tgts = [qT[q][:, h, :] for h in range(QH)] + [kT[q]]
            sws = []
            for tgt in tgts:  # issue all swap DMAs first (latency overlap)
                sw = rpool.tile([128, TQ], F16, tag="sw", bufs=5)
                nc.sync.dma_start(out=sw[0:64, :], in_=tgt[64:128, :])
                nc.sync.dma_start(out=sw[64:128, :], in_=tgt[0:64, :])
                sws.append(sw)
            for tgt, sw in zip(tgts, sws):
                tmp = rpool.tile([128, TQ], F16, tag="tmp")
                nc.vector.tensor_mul(tmp, tgt, cs)
                nc.vector.tensor_mul(sw, sw, sn)
                nc.vector.tensor_add(tgt, tmp, sw)

        def att(q):
            nj = 4 * (q + 1)
            L = nj // 2  # wide steps per head

            def blk(j):
                c0 = max(0, 128 * j - TQ * q)
                return c0, TQ - c0, j >= 4 * q

            def emit_swide(h, step):
                s_t = psum.tile([128, 2 * TQ], F32, tag="A", bufs=2,
                                name=f"s{q}_{h}_{step}")
                for u in (0, 1):
                    j = 2 * step + u
                    c0, N, diag = blk(j)
                    nc.tensor.matmul(
                        s_t[:, TQ * u:TQ * u + N],
                        kT[j // 4][:, 128 * (j % 4):128 * (j % 4) + 128],
                        qT[q][:, h, c0:TQ],
                        start=True, stop=not diag)
                    if diag:
                        nc.tensor.matmul(
                            s_t[:, TQ * u:TQ * u + 128], maskA, maskB,
                            start=False, stop=True)
                p_t = ppool.tile([128, 2 * TQ], F16, tag="p",
                                 name=f"p{q}_{h}_{step}")
                _, N1, _ = blk(2 * step + 1)
                if step == 0:
                    # split so the first den/o of this head waits less
                    nc.scalar.activation(
                        p_t[:, 0:TQ], s_t[:, 0:TQ], AF.Exp, scale=SCALE)
                    nc.scalar.activation(
                        p_t[:, TQ:TQ + N1], s_t[:, TQ:TQ + N1], AF.Exp,
                        scale=SCALE)
                else:
                    nc.scalar.activation(
                        p_t[:, 0:TQ + N1], s_t[:, 0:TQ + N1], AF.Exp,
                        scale=SCALE)
                return p_t

            def emit_deno(h, step, oacc, dacc, p_t):
                for u in (0, 1):
                    j = 2 * step + u
                    c0, N, _ = blk(j)
                    st, sp = (j == 0), (j == nj - 1)
                    nc.tensor.matmul(dacc[:, c0:c0 + N], ones_sq,
                                     p_t[:, TQ * u:TQ * u + N],
                                     start=st, stop=sp)
                    nc.tensor.matmul(oacc[:, c0:c0 + N],
                                     vS[j // 4][:, j % 4, :],
                                     p_t[:, TQ * u:TQ * u + N],
                                     start=st, stop=sp)

            def normalize(h, oacc, dacc):
                inv = rpool.tile([128, TQ], F32, tag="inv")
                nc.vector.reciprocal_approx_fast(out=inv, in_=dacc)
                nc.vector.tensor_mul(oT[q][:, h, :], oacc, inv)

            # flattened pipeline over both head pairs: den/o lags s/exp by one
            # position, so pair 2's prologue covers pair 1's epilogue.
            seq = [(h, step) for h in range(QH) for step in range(L)]
            state = {}  # h -> (oacc, dacc)
            pend = []   # [(h, step, p_t)]
            for (h, step) in seq:
                if step == 0:
                    oacc = psum.tile([128, TQ], F32, tag="B", bufs=2, name=f"o{q}_{h}")
                    dacc = psum.tile([128, TQ], F32, tag="C", bufs=2, name=f"d{q}_{h}")
                    state[h] = (oacc, dacc)
                p_t = emit_swide(h, step)
                pend.append((h, step, p_t))
                if len(pend) > 1:
                    ph, pstep, pp = pend.pop(0)
                    emit_deno(ph, pstep, *state[ph], pp)
                    if pstep == L - 1:
                        normalize(ph, *state[ph])
            ph, pstep, pp = pend.pop(0)
            emit_deno(ph, pstep, *state[ph], pp)
            normalize(ph, *state[ph])

        def wo_proj(q):
            out_t = outsb.tile([128, CK, TQ], F16, tag="o", name=f"out{q}")
            for co in range(CK):
                op = psum.tile([128, TQ], F32, tag="B", bufs=2, name=f"op{q}_{co}")
                for h in range(QH):
                    nc.tensor.matmul(op, wo_sb[:, co, h, :], oT[q][:, h, :],
                                     start=(h == 0), stop=(h == QH - 1))
                if co % 2 == 0:
                    nc.scalar.copy(out=out_t[:, co, :], in_=op)
                else:
                    nc.vector.tensor_copy(out=out_t[:, co, :], in_=op)
                if co % 4 == 3:  # stream output out as it completes
                    nc.gpsimd.dma_start(
                        out=outX[:, q, co - 3:co + 1, :],
                        in_=out_t[:, co - 3:co + 1, :])

        for q in range(NQ):
            proj(q)
            if q + 2 < NQ:
                x_load(q + 2)
            if q > 0:
                wo_proj(q - 1)
            att(q)
        wo_proj(NQ - 1)


_PERM = np.concatenate([np.arange(0, HD, 2), np.arange(1, HD, 2)])

PROFILE = False
LAST_EXEC_NS = None
LAST_RESULTS = None


def kernel(x, freqs_cos, freqs_sin, wq, wk, wv, wo):
    global LAST_EXEC_NS, LAST_RESULTS
    if "nc" not in _CACHE:
        _CACHE["nc"] = _build_nc()
    nc = _CACHE["nc"]

    x = np.asarray(x, dtype=np.float32)
    fc = np.asarray(freqs_cos, dtype=np.float32)
    fs = np.asarray(freqs_sin, dtype=np.float32)
    wq = np.asarray(wq, dtype=np.float32)
    wk = np.asarray(wk, dtype=np.float32)
    wv = np.asarray(wv, dtype=np.float32)
    wo = np.asarray(wo, dtype=np.float32)

    cosT = fc.T                                   # [64, T]
    sinT = fs.T
    cosX = np.ascontiguousarray(
        np.concatenate([cosT, cosT], axis=0)).astype(np.float16)   # [128, T]
    sinX = np.ascontiguousarray(
        np.concatenate([-sinT, sinT], axis=0)).astype(np.float16)

    in_maps = []
    for core in range(8):
        b, g = core // 4, core % 4
        # x: [p, q, k, t] = x[b, 512q+t, 128k+p]
        xb = x[b].astype(np.float16)              # [T, C]
        xQd = np.ascontiguousarray(
            xb.reshape(NQ, TQ, CK, 128).transpose(3, 0, 2, 1))
        # wq (rope-permuted rows): wqX[p, k, d] = wq_g.T[128k+p, d]
        wq_g = wq[512 * g:512 * (g + 1)].reshape(QH, HD, C)[:, _PERM, :]
        wqT = wq_g.reshape(QH * HD, C).T.astype(np.float16)        # [C, 512]
        wqX = np.ascontiguousarray(
            wqT.reshape(CK, 128, QH * HD).transpose(1, 0, 2))
        wkT = wk[HD * g:HD * (g + 1)][_PERM].T.astype(np.float16)  # [C, 128]
        wkX = np.ascontiguousarray(wkT.reshape(CK, 128, HD).transpose(1, 0, 2))
        wvT = wv[HD * g:HD * (g + 1)].T.astype(np.float16)
        wvX = np.ascontiguousarray(wvT.reshape(CK, 128, HD).transpose(1, 0, 2))
        # wo: woX[p, co, h, d] = wo[128co+d, 512g+128h+p]
        wo_g = wo[:, 512 * g:512 * (g + 1)].astype(np.float16)     # [C, 512]
        woX = np.ascontiguousarray(
            wo_g.reshape(CK, 128, QH, 128).transpose(3, 0, 2, 1))
        in_maps.append({
            "xQ": xQd, "wqX": wqX, "wkX": wkX, "wvX": wvX, "woX": woX,
            "cosX": cosX, "sinX": sinX,
        })

    res = run_bass_kernel_spmd(nc, in_maps, list(range(8)), trace=PROFILE)
    LAST_EXEC_NS = res.exec_time_ns
    LAST_RESULTS = res

    out = np.empty((B, T, C), dtype=np.float32)
    for b in range(B):
        acc = res.results[4 * b]["outX"].astype(np.float32)
        for g in range(1, 4):
            acc = acc + res.results[4 * b + g]["outX"]
        # outX[d, q, co, t] -> out[512q+t, 128co+d]
        out[b] = acc.transpose(1, 3, 2, 0).reshape(T, C)
    return out
